# revision 18
# baseline (speedup 1.0000x reference)
"""AFNO1D Trainium2 kernel (8 NeuronCores, SPMD over the token axis).

Math: the reference's DHT/flip/block-matmul pipeline folds exactly into:
  o1 = relu(x @ MA + flip_B(x) @ MC + b1)        (MA/MC dense 1024x1024, H1024 folded in)
  o2 = o1 . W2A + flip_B(o1) . W2C + b2          (per-block 128x128)
  z  = softshrink(o2, 0.01) @ (H128 / 2^24)      (per-block)
  out = z + x
flip_B is the batch permutation k -> (4-k)%4, handled in a batch-parity basis
(x0, x2 are flip-invariant; (x1+x3)/2 and (x1-x3)/2 are even/odd) so each layer
is one matmul stream per unit. |z| ~ 1e-8 * |x|, so bf16 is far inside the
tolerance; only the +x residual carries precision. Each core takes 512 of the
4096 tokens; nothing couples tokens, so no collectives.

Device layout: activations transposed [channel(128 part), chan_hi(8), rows],
rows r = slot*512 + tok with slot order [x0, x2, x1, x3]. Stages are fused per
block index m so the vector-engine epilogues of iteration m overlap the
matmuls of iteration m+1.
"""

import numpy as np
import ml_dtypes
from contextlib import ExitStack

import concourse.bass as bass
import concourse.tile as tile
import concourse.mybir as mybir
from concourse import bacc
from concourse.bass_utils import run_bass_kernel_spmd

NB, BS, HID = 8, 128, 1024
B, N = 4, 4096
NCORES = 8
TOK = N // NCORES            # 512 tokens per core
ROWS = B * TOK               # 2048 rows per core (4 slots x 512 tokens)
NUMEL = B * N * HID          # 2^24 (idht normalizes by total numel)
LAM = 0.01
RC = 512
HALF = 1024                  # two-slot slab

F32 = mybir.dt.float32
BF16 = mybir.dt.bfloat16


def _cas(n):
    idx = np.arange(n)
    ang = 2.0 * np.pi * np.outer(idx, idx) / n
    return np.cos(ang) + np.sin(ang)


def _flp(a):
    return np.roll(a[::-1], 1, axis=0)


def _fold_weights(w, H128):
    """w [2, nb, i, o] -> WA, WC [nb, i, o] float64 so that
    CM(x, w[0]) + CM(x, w[1]) = x . WA + flip_B(x) . WC  (per block)."""
    WA = np.zeros((NB, BS, BS))
    WC = np.zeros((NB, BS, BS))
    for j in range(2):
        y = w[j].astype(np.float64)
        Y = y @ H128
        yf = _flp(y)
        WA += 0.5 / NUMEL * np.einsum('ji,bio,ok->bjk', H128, Y + yf, H128)
        WC += 0.5 / NUMEL * (Y - yf) @ H128
    return WA, WC


def _prep_weights(w1, b1, w2, b2):
    H1024 = _cas(HID)
    H128 = _cas(BS)
    W1A, W1C = _fold_weights(w1, H128)
    W2A, W2C = _fold_weights(w2, H128)

    MA = np.zeros((HID, HID))
    MC = np.zeros((HID, HID))
    for b in range(NB):
        cols = slice(b * BS, (b + 1) * BS)
        MA[:, cols] = H1024[:, cols] @ W1A[b]
        MC[:, cols] = H1024[:, cols] @ W1C[b]
    Mp, Mm = MA + MC, MA - MC

    W2sum = W2A + W2C            # slots 0,1 (x0, x2: flip-invariant)
    W2ph = 0.5 * (W2A + W2C)     # e-path (s = o1[x1] + o1[x3])
    W2mh = 0.5 * (W2A - W2C)     # o-path (d = o1[x1] - o1[x3])

    def sb_m(M):  # [1024 in, 1024 out] -> [128, m(8), k(8), 128] (m-major chunks)
        t = M.reshape(NB, BS, NB, BS)          # [k_hi, k_lo, m_hi, m_lo]
        t = t.transpose(1, 2, 0, 3)            # [k_lo(part), m_hi, k_hi, m_lo]
        return np.ascontiguousarray(t.astype(ml_dtypes.bfloat16))

    def sb_blk(W):  # [nb, i, o] -> [128, nb, o]
        return np.ascontiguousarray(W.transpose(1, 0, 2).astype(ml_dtypes.bfloat16))

    ident = np.eye(BS)
    return {
        "Mp": sb_m(Mp), "Mm": sb_m(Mm),
        "W2sum": sb_blk(W2sum), "W2ph": sb_blk(W2ph), "W2mh": sb_blk(W2mh),
        "H128s": np.ascontiguousarray((H128 / NUMEL).astype(ml_dtypes.bfloat16)),
        "Ident": np.ascontiguousarray(ident.astype(ml_dtypes.bfloat16)),
        "b1": np.ascontiguousarray(b1[0].astype(np.float32).T),   # [128, 8]
        "b2": np.ascontiguousarray(b2[0].astype(np.float32).T),   # [128, 8]
    }


def build_nc():
    nc = bacc.Bacc("TRN2", target_bir_lowering=False, debug=False)

    xr_ext = [nc.declare_dram_parameter(f"xr{u}", [BS, NB, RC], BF16, isOutput=False)
              for u in range(4)]
    xeo_ext = nc.declare_dram_parameter("xeo", [BS, NB, HALF], BF16, isOutput=False)
    mp_ext = nc.declare_dram_parameter("Mp", [BS, NB, NB, BS], BF16, isOutput=False)
    mm_ext = nc.declare_dram_parameter("Mm", [BS, NB, NB, BS], BF16, isOutput=False)
    w2s_ext = nc.declare_dram_parameter("W2sum", [BS, NB, BS], BF16, isOutput=False)
    w2p_ext = nc.declare_dram_parameter("W2ph", [BS, NB, BS], BF16, isOutput=False)
    w2m_ext = nc.declare_dram_parameter("W2mh", [BS, NB, BS], BF16, isOutput=False)
    h_ext = nc.declare_dram_parameter("H128s", [BS, BS], BF16, isOutput=False)
    id_ext = nc.declare_dram_parameter("Ident", [BS, BS], BF16, isOutput=False)
    b1_ext = nc.declare_dram_parameter("b1", [BS, NB], F32, isOutput=False)
    b2_ext = nc.declare_dram_parameter("b2", [BS, NB], F32, isOutput=False)
    out_ext = nc.declare_dram_parameter("out", [BS, NB, ROWS], BF16, isOutput=True)

    RELU = mybir.ActivationFunctionType.Relu
    IDENT = mybir.ActivationFunctionType.Identity
    ADD = mybir.AluOpType.add
    SUB = mybir.AluOpType.subtract
    MAX = mybir.AluOpType.max
    MIN = mybir.AluOpType.min
    MULT = mybir.AluOpType.mult
    from bass_rust import add_dep_helper

    with tile.TileContext(nc) as tc:
        with ExitStack() as ctx:
            wpool = ctx.enter_context(tc.tile_pool(name="w", bufs=1))
            apool = ctx.enter_context(tc.tile_pool(name="act", bufs=1))
            tpool = ctx.enter_context(tc.tile_pool(name="tmp", bufs=2))
            opool = ctx.enter_context(tc.tile_pool(name="outb", bufs=2))
            ppool = ctx.enter_context(tc.tile_pool(name="ps", bufs=4, space="PSUM"))

            # ---- resident tensors, split per-chunk for fine-grained deps ----
            xr = [apool.tile([BS, NB, RC], BF16, name=f"xr_{u}") for u in range(4)]
            xeo = apool.tile([BS, NB, HALF], BF16)
            Mp = [wpool.tile([BS, NB, BS], BF16, name=f"Mp_{m}") for m in range(NB)]
            Mm = [wpool.tile([BS, NB, BS], BF16, name=f"Mm_{m}") for m in range(NB)]

            # wave 0: what the first iteration needs, most-urgent first
            nc.sync.dma_start(xr[0][:], xr_ext[0][:])
            nc.sync.dma_start(Mp[0][:], mp_ext[:, 0])
            nc.sync.dma_start(xr[1][:], xr_ext[1][:])
            nc.sync.dma_start(xeo[:], xeo_ext[:])
            nc.sync.dma_start(Mm[0][:], mm_ext[:, 0])
            b1 = wpool.tile([BS, NB], F32)
            nc.sync.dma_start(b1[:], b1_ext[:])
            b2 = wpool.tile([BS, NB], F32)
            nc.sync.dma_start(b2[:], b2_ext[:])
            # later waves gated on compute progress (below)
            wave1 = []
            W2s = wpool.tile([BS, NB, BS], BF16)
            wave1.append((nc.sync.dma_start(W2s[:], w2s_ext[:]), None))
            W2p = wpool.tile([BS, NB, BS], BF16)
            wave1.append((nc.sync.dma_start(W2p[:], w2p_ext[:]), None))
            W2m = wpool.tile([BS, NB, BS], BF16)
            wave1.append((nc.sync.dma_start(W2m[:], w2m_ext[:]), None))
            H128s = wpool.tile([BS, BS], BF16)
            wave1.append((nc.sync.dma_start(H128s[:], h_ext[:]), None))
            Ident = wpool.tile([BS, BS], BF16)
            wave1.append((nc.sync.dma_start(Ident[:], id_ext[:]), None))
            wave1.append((nc.sync.dma_start(xr[2][:], xr_ext[2][:]), None))
            wave1.append((nc.sync.dma_start(xr[3][:], xr_ext[3][:]), None))
            wavem = {}
            for m in range(1, NB):
                wavem[m] = [nc.sync.dma_start(Mp[m][:], mp_ext[:, m]),
                            nc.sync.dma_start(Mm[m][:], mm_ext[:, m])]

            b2n = wpool.tile([BS, NB], F32)
            nc.vector.tensor_scalar(b2n[:], b2[:], -1.0, None, MULT)

            o1 = apool.tile([BS, NB, ROWS], BF16)
            sd = apool.tile([BS, NB, HALF], BF16)   # s | d per block

            def sl(lo, n=RC):
                return bass.ds(lo, n)

            state = {}   # per-block carried tiles

            def l2_matmuls(b):
                pc = ppool.tile([BS, HALF], F32, tag="ps", name=f"pc_{b}")
                pd = ppool.tile([BS, HALF], F32, tag="ps", name=f"pd_{b}")
                nc.tensor.matmul(pc[:, 0:RC], W2s[:, b], o1[:, b, sl(0)],
                                 start=True, stop=True)
                nc.tensor.matmul(pc[:, RC:HALF], W2s[:, b], o1[:, b, sl(RC)],
                                 start=True, stop=True)
                nc.tensor.matmul(pd[:, 0:RC], W2p[:, b], sd[:, b, 0:RC],
                                 start=True, stop=True)
                nc.tensor.matmul(pd[:, RC:HALF], W2m[:, b], sd[:, b, RC:HALF],
                                 start=True, stop=True)
                state[b] = (pc, pd)

            def l2_epilogue(b):
                pc, pd = state.pop(b)
                zt = opool.tile([BS, ROWS], BF16, tag="zt", name=f"zt_{b}")
                # slots 2,3 first: longer chain, F needs it later
                ob2 = tpool.tile([BS, HALF], F32, tag="ob2", name=f"ob2_{b}")
                nc.scalar.activation(ob2[:, 0:RC], pd[:, RC:HALF], IDENT,
                                     bias=b2[:, b:b + 1])
                nc.scalar.activation(ob2[:, RC:HALF], pd[:, RC:HALF], IDENT,
                                     bias=b2n[:, b:b + 1])
                v13 = tpool.tile([BS, HALF], BF16, tag="v13", name=f"v13_{b}")
                nc.vector.tensor_tensor(v13[:, 0:RC], pd[:, 0:RC], ob2[:, 0:RC], ADD)
                nc.vector.tensor_tensor(v13[:, RC:HALF], pd[:, 0:RC],
                                        ob2[:, RC:HALF], SUB)
                t13c = tpool.tile([BS, HALF], BF16, tag="t13c", name=f"t13c_{b}")
                nc.vector.tensor_scalar(t13c[:], v13[:], -LAM, LAM, MAX, MIN)
                nc.gpsimd.tensor_tensor(zt[:, HALF:ROWS], v13[:], t13c[:], SUB)
                # slots 0,1
                v02 = tpool.tile([BS, HALF], BF16, tag="v02", name=f"v02_{b}")
                nc.scalar.activation(v02[:], pc[:], IDENT, bias=b2[:, b:b + 1])
                t02 = tpool.tile([BS, HALF], BF16, tag="t02", name=f"t02_{b}")
                nc.vector.tensor_scalar(t02[:], v02[:], -LAM, LAM, MAX, MIN)
                nc.vector.tensor_tensor(zt[:, 0:HALF], v02[:], t02[:], SUB)
                state[b] = zt

            def stage_f(b):
                zt = state.pop(b)
                ob = opool.tile([BS, ROWS], BF16, tag="ob", name=f"ob_{b}")
                for h in (0, HALF):
                    pf = ppool.tile([BS, HALF], F32, tag="ps", name=f"pf_{b}_{h}")
                    nc.tensor.matmul(pf[:, 0:RC], H128s[:], zt[:, sl(h)],
                                     start=True, stop=False)
                    nc.tensor.matmul(pf[:, RC:HALF], H128s[:], zt[:, sl(h + RC)],
                                     start=True, stop=False)
                    nc.tensor.matmul(pf[:, 0:RC], Ident[:],
                                     xr[h // RC][:, b, :], start=False, stop=True)
                    nc.tensor.matmul(pf[:, RC:HALF], Ident[:],
                                     xr[h // RC + 1][:, b, :], start=False, stop=True)
                    if h == 0:
                        nc.scalar.activation(ob[:, 0:HALF], pf[:], IDENT)
                    else:
                        nc.vector.tensor_copy(ob[:, HALF:ROWS], pf[:])
                nc.sync.dma_start(out_ext[:, b, :], ob[:])

            first_mm = None
            for m in range(NB):
                # --- L2 (block m-1): matmuls then epilogue, queued early so
                #     the zt chain completes while psA(m) streams on PE ---
                if m > 0:
                    l2_matmuls(m - 1)
                    l2_epilogue(m - 1)

                # --- L01 slab A (slots 0,1) ---
                psA = ppool.tile([BS, HALF], F32, tag="ps", name=f"psA_{m}")
                for k in range(NB):
                    st, sp = (k == 0), (k == NB - 1)
                    mm = nc.tensor.matmul(psA[:, 0:RC], Mp[m][:, k], xr[0][:, k, :],
                                          start=st, stop=sp)
                    if first_mm is None:
                        first_mm = mm
                        for d, _ in wave1:
                            add_dep_helper(d.ins, mm.ins, reason="dma staging w1")
                    nc.tensor.matmul(psA[:, RC:HALF], Mp[m][:, k], xr[1][:, k, :],
                                     start=st, stop=sp)
                if m + 2 in wavem:
                    for d in wavem[m + 2]:
                        add_dep_helper(d.ins, mm.ins, reason="dma staging wm")
                nc.scalar.activation(o1[:, m, sl(0, HALF)], psA[:], RELU,
                                     bias=b1[:, m:m + 1])

                # --- F for block m-1 ---
                if m > 0:
                    stage_f(m - 1)

                # --- L01 slab B (e|o) + its epilogue ---
                psB = ppool.tile([BS, HALF], F32, tag="ps", name=f"psB_{m}")
                for k in range(NB):
                    st, sp = (k == 0), (k == NB - 1)
                    nc.tensor.matmul(psB[:, 0:RC], Mp[m][:, k], xeo[:, k, sl(0)],
                                     start=st, stop=sp)
                    nc.tensor.matmul(psB[:, RC:HALF], Mm[m][:, k], xeo[:, k, sl(RC)],
                                     start=st, stop=sp)
                osb = tpool.tile([BS, RC], F32, tag="osb", name=f"osb_{m}")
                nc.scalar.activation(osb[:], psB[:, RC:HALF], IDENT)
                t13 = tpool.tile([BS, HALF], F32, tag="t13", name=f"t13_{m}")
                nc.vector.tensor_tensor(t13[:, 0:RC], psB[:, 0:RC], osb[:], ADD)
                nc.vector.tensor_tensor(t13[:, RC:HALF], psB[:, 0:RC], osb[:], SUB)
                nc.scalar.activation(o1[:, m, sl(HALF, HALF)], t13[:], RELU,
                                     bias=b1[:, m:m + 1])
                nc.vector.tensor_tensor(sd[:, m, 0:RC], o1[:, m, sl(HALF)],
                                        o1[:, m, sl(HALF + RC)], ADD)
                nc.vector.tensor_tensor(sd[:, m, RC:HALF], o1[:, m, sl(HALF)],
                                        o1[:, m, sl(HALF + RC)], SUB)

            # flush the last block
            l2_matmuls(NB - 1)
            l2_epilogue(NB - 1)
            stage_f(NB - 1)

    nc.compile()
    return nc


_CACHED = {}


def _get_nc():
    if "nc" not in _CACHED:
        _CACHED["nc"] = build_nc()
    return _CACHED["nc"]


def _make_in_maps(x, w1, b1, w2, b2):
    wd = _prep_weights(w1, b1, w2, b2)

    xf = np.asarray(x, dtype=np.float32)
    slots = np.empty((B, N, HID), np.float32)   # row-slot order x0, x2, x1, x3
    slots[0] = xf[0]
    slots[1] = xf[2]
    slots[2] = xf[1]
    slots[3] = xf[3]
    eo = np.empty((2, N, HID), np.float32)
    eo[0] = 0.5 * (xf[1] + xf[3])
    eo[1] = 0.5 * (xf[1] - xf[3])

    def to_dev(a, nrows):   # [..., 1024] -> [128, 8, nrows] bf16
        aT = a.reshape(nrows, HID).T
        return np.ascontiguousarray(
            aT.reshape(NB, BS, nrows).transpose(1, 0, 2).astype(ml_dtypes.bfloat16))

    in_maps = []
    for c in range(NCORES):
        ts = slice(c * TOK, (c + 1) * TOK)
        m = {f"xr{u}": to_dev(slots[u, ts, :], TOK) for u in range(4)}
        m["xeo"] = to_dev(eo[:, ts, :], HALF)
        m.update(wd)
        in_maps.append(m)
    return in_maps


def kernel(x, w1, b1, w2, b2):
    out_dtype = x.dtype
    in_maps = _make_in_maps(x, w1, b1, w2, b2)
    nc = _get_nc()
    res = run_bass_kernel_spmd(nc, in_maps, core_ids=list(range(NCORES)))

    out = np.empty((B, N, HID), np.float32)
    for c in range(NCORES):
        ob = np.asarray(res.results[c]["out"], dtype=np.float32)  # [128, 8, 2048]
        full = ob.transpose(1, 0, 2).reshape(HID, ROWS).T         # [2048, 1024]
        full = full.reshape(B, TOK, HID)                          # slot-major
        ts = slice(c * TOK, (c + 1) * TOK)
        out[0, ts] = full[0]
        out[2, ts] = full[1]
        out[1, ts] = full[2]
        out[3, ts] = full[3]
    return out.astype(out_dtype)
